# revision 27
# baseline (speedup 1.0000x reference)
"""Trainium2 Bass kernel for AuxiliaryGovernedAttention.

Math (see reference):
  q       = hidden @ W_q.T / sqrt(64)                    [B,S,D]
  scores  = q @ aux_keys.T + log(reliability + 1e-10)    [B,S,NS]
  attn    = softmax(scores, -1)
  aux_out = attn @ aux_values                            [B,S,H]
  avg_w   = mean_h(primary_attention_weights)            [B,S,S]
  entropy = -sum(avg_w * log(avg_w + 1e-10), -1)         [B,S]
  gate    = sigmoid(w1*entropy + b); veto <0.5 -> 0; >2.0 -> min(gate, 0.8)
  out     = primary_attention_output + gate * aux_out

Sharding: flatten (B,S) -> 4096 query rows; core c owns rows
[c*512, (c+1)*512). Small tensors replicated; no collectives.

The dominant cost is streaming primary_attention_weights. It is shipped
pre-scaled by 2^13 in fp8e4m3 (33.5 MB/core instead of 134 MB f32); the
32-head sum runs on the TensorEngine as DoubleRow fp8 matmuls against a
stacked pair-identity (2 heads per matmul, 0.5 cyc/row) accumulating
exactly in PSUM f32. The accumulator is split into two double-buffered
1024-wide halves so the entropy drain of one half overlaps the
accumulation of the next and the PE never stalls at block boundaries.
Entropy = Act Ln + DVE multiply/reduce; the 2^13 scale folds into the
Ln scale and gate constants. The gate sigmoid is a host-fitted cubic
evaluated on the DVE (no Act Exp in steady state, so the activation
table stays on natural_log and never reloads mid-stream). hidden/W_q
ride fp8 (DoubleRow q-projection); pao rides bf16; the output is
stored bf16 and upcast on host.

Ring assignment: paw stream owns the SP (sync) HWDGE queue; all loads
ride the ACT queue; stores ride SWDGE (gpsimd) so nothing ever queues
behind the paw stream. The aux path (softmax + attn @ aux_values,
scaled by 1/sum on drain) is computed in the shadow of the paw stream;
the gate is applied per block at combine time (Act scale-mul + DVE add).
"""

import sys
from contextlib import ExitStack

import ml_dtypes
import numpy as np

sys.path.insert(0, "/opt/trn_rl_repo")

import concourse.mybir as mybir
import concourse.tile as tile
from concourse import bacc
from concourse.bass_utils import run_bass_kernel_spmd

F32 = mybir.dt.float32
BF16 = mybir.dt.bfloat16
FP8 = mybir.dt.float8e4
AF = mybir.ActivationFunctionType
ALU = mybir.AluOpType
DR = mybir.MatmulPerfMode.DoubleRow

B, S, H, NH, NS, D = 2, 2048, 4096, 32, 100, 64
NCORES = 8
ROWS = (B * S) // NCORES    # 512 query rows per core
BLK = 128                   # queries per block (partition dim)
NBLK = ROWS // BLK          # 4 blocks per core
HP = NH // 2                # 16 head pairs per block
KT2 = H // 256              # 16 k-tile pairs for the q projection
HCH = 512                   # matmul free chunk (one PSUM bank)
NHCH = H // HCH             # 8 aux chunks
SH = S // 2                 # entropy accumulator half width (1024)
CCH = 1024                  # final combine/store chunk
NCCH = H // CCH             # 4 chunks
PAW_SCALE = 8192.0          # 2^13: fp8 paw values ~ U(0, 8]
ACC_INV = 1.0 / (NH * PAW_SCALE)   # 2^-18: acc -> avg_w
# entropy thresholds in r = -2^18 * entropy terms
R_TAU_LOW = -0.5 * NH * PAW_SCALE   # ent < 0.5  <=>  r > -131072
R_TAU_HIGH = -2.0 * NH * PAW_SCALE  # ent > 2.0  <=>  r < -524288

_GRAPH_CACHE = {}


def build_graph():
    nc = bacc.Bacc()
    paw_d = nc.declare_dram_parameter("paw", [NBLK * HP, BLK, 2 * S], FP8, isOutput=False)
    hst_d = nc.declare_dram_parameter("hst", [128, KT2 * 2 * ROWS], FP8, isOutput=False)
    pao_d = nc.declare_dram_parameter("pao", [ROWS, H], BF16, isOutput=False)
    wqt_d = nc.declare_dram_parameter("wqt", [128, KT2 * 2 * D], FP8, isOutput=False)
    akt_d = nc.declare_dram_parameter("akt", [D, NS], BF16, isOutput=False)
    av_d = nc.declare_dram_parameter("av", [NS, H], FP8, isOutput=False)
    idp_d = nc.declare_dram_parameter("idp", [128, 2 * 128], FP8, isOutput=False)
    idb_d = nc.declare_dram_parameter("idb", [128, 128], BF16, isOutput=False)
    cst_d = nc.declare_dram_parameter("cst", [128, 6 + NS], F32, isOutput=False)
    out_d = nc.declare_dram_parameter("out", [ROWS, H], BF16, isOutput=True)

    with ExitStack() as ctx:
        tc = ctx.enter_context(tile.TileContext(nc))
        const_p = ctx.enter_context(tc.tile_pool(name="const", bufs=1))
        paw_p = ctx.enter_context(tc.tile_pool(name="paw", bufs=23))
        pao_p = ctx.enter_context(tc.tile_pool(name="pao", bufs=1))
        out_p = ctx.enter_context(tc.tile_pool(name="out", bufs=2))
        ln_p = ctx.enter_context(tc.tile_pool(name="ln", bufs=2))
        small_p = ctx.enter_context(tc.tile_pool(name="small", bufs=2))
        mm_ps = ctx.enter_context(tc.tile_pool(name="mm_ps", bufs=4, space="PSUM"))
        acc_ps = ctx.enter_context(tc.tile_pool(name="acc_ps", bufs=2, space="PSUM"))

        # ---- one-time constants (ACT HWDGE ring); idp/cst first so the
        # head-sum for block 0 can start as soon as paw tiles land.
        idp = const_p.tile([128, 2, 128], FP8, tag="idp")
        nc.scalar.dma_start(out=idp[:], in_=idp_d[:])
        cst = const_p.tile([128, 6 + NS], F32, tag="cst")
        nc.scalar.dma_start(out=cst[:], in_=cst_d[:])
        idb = const_p.tile([128, 128], BF16, tag="idb")
        nc.scalar.dma_start(out=idb[:], in_=idb_d[:])
        akt = const_p.tile([D, NS], BF16, tag="akt")
        nc.scalar.dma_start(out=akt[:], in_=akt_d[:])
        # av fits the pre-paw window on the Act ring; the pao loads are
        # emitted late (per block, into Act's mid-stream slack) so the SP
        # queue's hst chunks don't compete with them for DMA engines, and
        # SWDGE carries nothing but stores.
        av = const_p.tile([NS, H], FP8, tag="av")
        nc.scalar.dma_start(out=av[:], in_=av_d[:])
        pao_all = [
            pao_p.tile([BLK, H], BF16, tag=f"pao{b}", name=f"pao{b}")
            for b in range(NBLK)
        ]

        def emit_pao_load(b):
            nc.scalar.dma_start(
                out=pao_all[b][:], in_=pao_d[b * BLK : (b + 1) * BLK, :]
            )

        # ---- SP ring: wqt + hst first (q-proj needs them in the first
        # ~8us, and every other ring starves once the paw stream saturates
        # the DMA engines), then the paw pair-tile stream, in order ----
        wqt = const_p.tile([128, KT2, 2, D], FP8, tag="wqt")
        nc.sync.dma_start(out=wqt[:], in_=wqt_d[:])
        hst_chunks = []
        for q in range(4):
            hstc = paw_p.tile(
                [128, KT2 // 4, 2, ROWS], FP8, tag="hstc", bufs=2, name=f"hstc{q}"
            )
            nc.sync.dma_start(
                out=hstc[:],
                in_=hst_d[:, q * (KT2 * 2 * ROWS // 4) : (q + 1) * (KT2 * 2 * ROWS // 4)],
            )
            hst_chunks.append(hstc)
        pw_tiles = [[None] * HP for _ in range(NBLK)]
        for b in range(NBLK):
            for hp in range(HP):
                pw = paw_p.tile([BLK, 2, S], FP8, tag="pw")
                nc.sync.dma_start(out=pw[:], in_=paw_d[b * HP + hp])
                pw_tiles[b][hp] = pw

        def emit_headsum(b):
            """DoubleRow pair-identity head-sum for block b into two
            1024-wide PSUM halves (acc tag, bufs=2): the entropy drain
            of block b overlaps the accumulation of block b+1."""
            accs = [
                acc_ps.tile([BLK, SH], F32, tag="acc", name=f"acc{b}_{h}")
                for h in range(2)
            ]
            for hp in range(HP):
                pw = pw_tiles[b][hp]
                for c in range(4):
                    nc.tensor.matmul(
                        accs[c // 2][:, (c % 2) * HCH : (c % 2 + 1) * HCH],
                        lhsT=idp[:],
                        rhs=pw[:, :, c * HCH : (c + 1) * HCH],
                        start=(hp == 0),
                        stop=(hp == HP - 1),
                        perf_mode=DR,
                    )
            return accs

        def emit_entropy_gate(b, accs):
            # entropy halves: ln_t = Ln(acc/2^18 + 1e-10)
            # rh[h] = sum(acc * ln_t);  r = rh[0]+rh[1] = -2^18 * entropy
            rh = small_p.tile([BLK, 2], F32, tag="rh", name="rh")
            for h in range(2):
                ln_t = ln_p.tile([BLK, SH], BF16, tag="ln", name="ln_t")
                nc.scalar.activation(
                    ln_t[:], accs[h][:], AF.Ln, bias=cst[:, 4:5], scale=ACC_INV
                )
                nc.vector.tensor_mul(ln_t[:], accs[h][:], ln_t[:])
                nc.vector.reduce_sum(
                    rh[:, h : h + 1], ln_t[:], axis=mybir.AxisListType.X
                )
            r_t = small_p.tile([BLK, 1], F32, tag="r")
            nc.vector.tensor_add(r_t[:], rh[:, 0:1], rh[:, 1:2])

            # gate = sigmoid(w1*ent + bias) via host-fitted cubic in r
            # (poly coeffs in cst[:,0:4]; exact veto handling below)
            g0 = small_p.tile([BLK, 1], F32, tag="g0")
            nc.vector.tensor_scalar(
                g0[:], r_t[:], cst[:, 3:4], cst[:, 2:3], op0=ALU.mult, op1=ALU.add
            )
            nc.vector.tensor_mul(g0[:], g0[:], r_t[:])
            nc.vector.tensor_scalar_add(g0[:], g0[:], cst[:, 1:2])
            nc.vector.tensor_mul(g0[:], g0[:], r_t[:])
            nc.vector.tensor_scalar_add(g0[:], g0[:], cst[:, 0:1])
            # veto: ent<0.5 (r>-131072) -> 0 ; ent>2.0 (r<-524288) -> min(g,0.8)
            mlo = small_p.tile([BLK, 1], F32, tag="mlo")
            nc.vector.tensor_scalar(mlo[:], r_t[:], R_TAU_LOW, None, op0=ALU.is_le)
            mhi = small_p.tile([BLK, 1], F32, tag="mhi")
            nc.vector.tensor_scalar(mhi[:], r_t[:], R_TAU_HIGH, None, op0=ALU.is_lt)
            exc = small_p.tile([BLK, 1], F32, tag="exc")
            nc.vector.tensor_scalar(
                exc[:], g0[:], 0.8, 0.0, op0=ALU.subtract, op1=ALU.max
            )
            nc.vector.tensor_mul(exc[:], exc[:], mhi[:])
            nc.vector.tensor_sub(g0[:], g0[:], exc[:])
            nc.vector.tensor_mul(g0[:], g0[:], mlo[:])
            return g0

        def emit_aux(b):
            # aux_out (pre-gate, scaled by 1/sum)
            for j in range(NHCH):
                ax = mm_ps.tile([BLK, HCH], F32, tag="scratch", name="ax")
                nc.tensor.matmul(
                    ax[:], lhsT=pt_all[b][:], rhs=av[:, j * HCH : (j + 1) * HCH]
                )
                nc.scalar.activation(
                    axs_all[b][:, j * HCH : (j + 1) * HCH], ax[:], AF.Copy,
                    scale=inv[:, b : b + 1],
                )

        def emit_combine(b, g0):
            # combine + store (SWDGE ring), chunked to shorten the tail:
            # Act applies the gate scale, DVE adds the residual.
            r0 = b * BLK
            out_t = out_p.tile([BLK, H], BF16, tag="out", name="out_t")
            for j in range(NCCH):
                j0, j1 = j * CCH, (j + 1) * CCH
                gxj = out_p.tile([BLK, CCH], BF16, tag="gx", name="gxj")
                if j % 2 == 0:
                    nc.scalar.activation(
                        gxj[:], axs_all[b][:, j0:j1], AF.Copy, scale=g0[:]
                    )
                else:
                    nc.vector.tensor_scalar(
                        gxj[:], axs_all[b][:, j0:j1], g0[:], None, op0=ALU.mult
                    )
                nc.vector.tensor_add(out_t[:, j0:j1], gxj[:], pao_all[b][:, j0:j1])
            nc.gpsimd.dma_start(out=out_d[r0 : r0 + BLK, :], in_=out_t[:])

        # ---- prologue: q projection qT[64, 512] via fp8 DoubleRow ----
        qt_psum = mm_ps.tile([D, ROWS], F32, tag="scratch")
        for k in range(KT2):
            nc.tensor.matmul(
                qt_psum[:],
                lhsT=wqt[:, k],
                rhs=hst_chunks[k // (KT2 // 4)][:, k % (KT2 // 4)],
                start=(k == 0),
                stop=(k == KT2 - 1),
                perf_mode=DR,
            )
        qt_sb = const_p.tile([D, ROWS], BF16, tag="qt_sb")
        nc.scalar.copy(qt_sb[:], qt_psum[:])

        # ---- softmax numerators (transposed) + 1/sum for all blocks ----
        ssum = const_p.tile([128, NBLK], F32, tag="ssum")
        inv = const_p.tile([128, NBLK], F32, tag="inv")
        pt_all = []
        for b in range(NBLK):
            r0 = b * BLK
            sc_psum = mm_ps.tile([BLK, NS], F32, tag="scratch", name="sc_psum")
            nc.tensor.matmul(sc_psum[:], lhsT=qt_sb[:, r0 : r0 + BLK], rhs=akt[:])
            sc_sb = small_p.tile([BLK, NS], F32, tag="sc_sb")
            nc.vector.tensor_add(sc_sb[:], sc_psum[:], cst[:, 6 : 6 + NS])
            p_t = small_p.tile([BLK, NS], BF16, tag="p")
            nc.scalar.activation(
                p_t[:], sc_sb[:], AF.Exp, bias=cst[:, 5:6],
                accum_out=ssum[:, b : b + 1],
            )
            nc.vector.reciprocal(inv[:, b : b + 1], ssum[:, b : b + 1])
            nc.vector.tensor_scalar_mul(inv[:, b : b + 1], inv[:, b : b + 1], 1.0 / 64.0)
            pt_psum = mm_ps.tile([NS, BLK], BF16, tag="scratch", name="pt_psum")
            nc.tensor.transpose(pt_psum[:], p_t[:], idb[:])
            ptb = const_p.tile([NS, BLK], BF16, tag=f"pt{b}", name=f"ptb{b}")
            nc.scalar.copy(ptb[:], pt_psum[:])
            pt_all.append(ptb)

        axs_all = [
            const_p.tile([BLK, H], BF16, tag=f"axs{b}", name=f"axs{b}")
            for b in range(NBLK)
        ]

        # aux for the last block is hoisted here so its Act drains land
        # mid-stream, keeping the end-of-kernel tail to entropy + combine.
        emit_aux(NBLK - 1)
        emit_pao_load(0)

        for b in range(NBLK):
            accs = emit_headsum(b)
            g0_b = emit_entropy_gate(b, accs)
            if b + 1 < NBLK:
                emit_pao_load(b + 1)
            if b < NBLK - 1:
                emit_aux(b)
            emit_combine(b, g0_b)

    nc.compile()
    return nc


def _get_graph():
    key = "g"
    if key not in _GRAPH_CACHE:
        _GRAPH_CACHE[key] = build_graph()
    return _GRAPH_CACHE[key]


def _sigmoid_poly_coeffs(w1, gb):
    """Cubic fit of gate0(r) = sigmoid(-w1*2^-18*r + gb) over the z range
    the veto logic actually exposes (|poly-sigmoid| <~ 6e-3, and the gate
    multiplies an aux term that is ~0.3% of the output)."""
    z = np.linspace(-1.3, 3.0, 2001)
    a = -w1 * ACC_INV
    if abs(a) < 1e-30:
        return np.array([1.0 / (1.0 + np.exp(-gb)), 0.0, 0.0, 0.0])
    r = (z - gb) / a
    g = 1.0 / (1.0 + np.exp(-z))
    c3, c2, c1, c0 = np.polyfit(r, g, 3)
    return np.array([c0, c1, c2, c3], dtype=np.float64)


def _make_in_maps(inputs):
    bf = ml_dtypes.bfloat16
    f8 = ml_dtypes.float8_e4m3

    hs = np.asarray(inputs["hidden_states"], dtype=np.float32).reshape(B * S, H)
    pao = np.asarray(inputs["primary_attention_output"], dtype=np.float32).reshape(
        B * S, H
    )
    paw = np.asarray(inputs["primary_attention_weights"], dtype=np.float32)
    rel = np.asarray(inputs["reliability"], dtype=np.float32)
    wq = np.asarray(inputs["W_q"], dtype=np.float32)
    ak = np.asarray(inputs["aux_keys"], dtype=np.float32)
    av = np.asarray(inputs["aux_values"], dtype=np.float32)
    w1 = float(np.asarray(inputs["gate_w1"]))
    gb = float(np.asarray(inputs["gate_bias"]))

    # paw scaled to fp8 once for the full tensor, then sliced per core
    paw8 = (paw * PAW_SCALE).astype(f8)

    # W_q.T * 8 packed as [p, kt2, i, d] DoubleRow k-tile pairs; the *8
    # (instead of /8) is compensated by akt = aux_keys.T / 64.
    wqt = (
        (wq * 8.0).T.astype(f8)
        .reshape(KT2, 2, 128, D).transpose(2, 0, 1, 3).reshape(128, KT2 * 2 * D)
    )
    wqt = np.ascontiguousarray(wqt)
    akt = np.ascontiguousarray(ak.T / 64.0).astype(bf)
    avc = np.ascontiguousarray(av * 64.0).astype(f8)

    # stacked pair-identity for the DoubleRow head-sum
    idp = np.zeros((128, 2, 128), dtype=f8)
    ii = np.arange(128)
    idp[ii, 0, ii] = 1.0
    idp[ii, 1, ii] = 1.0
    idp = idp.reshape(128, 256)

    poly = _sigmoid_poly_coeffs(w1, gb)
    cst = np.zeros((128, 6 + NS), dtype=np.float32)
    cst[:, 0:4] = poly[None, :]   # gate cubic c0..c3
    cst[:, 4] = 1e-10             # Ln bias
    cst[:, 5] = 0.0               # Exp bias (scores)
    cst[:, 6:] = np.log(rel + 1e-10)[None, :]

    idb = np.eye(128, dtype=bf)

    in_maps = []
    for c in range(NCORES):
        b = c // (NCORES // B)
        s0 = (c % (NCORES // B)) * ROWS
        rows = slice(c * ROWS, (c + 1) * ROWS)

        # paw pair-tiles: [NH, ROWS, S] -> [blk*HP+hp, r, (i, s)]
        pc = paw8[b, :, s0 : s0 + ROWS, :]
        pc = (
            pc.reshape(HP, 2, NBLK, BLK, S)
            .transpose(2, 0, 3, 1, 4)
            .reshape(NBLK * HP, BLK, 2 * S)
        )

        # hidden rows, transposed, packed as [p, kt2, i, r]
        hc = (
            hs[rows].T.astype(f8)
            .reshape(KT2, 2, 128, ROWS).transpose(2, 0, 1, 3)
            .reshape(128, KT2 * 2 * ROWS)
        )

        in_maps.append(
            {
                "paw": np.ascontiguousarray(pc),
                "hst": np.ascontiguousarray(hc),
                "pao": np.ascontiguousarray(pao[rows]).astype(bf),
                "wqt": wqt,
                "akt": akt,
                "av": avc,
                "idp": idp,
                "idb": idb,
                "cst": cst,
            }
        )
    return in_maps


def _gather_out(res):
    out = np.concatenate(
        [res.results[i]["out"].astype(np.float32) for i in range(NCORES)], axis=0
    )
    return np.ascontiguousarray(out.reshape(B, S, H))


def kernel(**inputs) -> np.ndarray:
    nc = _get_graph()
    in_maps = _make_in_maps(inputs)
    res = run_bass_kernel_spmd(nc, in_maps, list(range(NCORES)))
    return _gather_out(res)


def kernel_traced(inputs, **kw):
    """test-harness entry: returns (output, BassKernelResults)."""
    nc = _get_graph()
    in_maps = _make_in_maps(inputs)
    res = run_bass_kernel_spmd(nc, in_maps, list(range(NCORES)), trace=True, **kw)
    return _gather_out(res), res
